# revision 1
# baseline (speedup 1.0000x reference)
"""Trainium2 Bass kernel for Grossberg dynamics (batched gated 17x17 matvecs).

dS/dt = (-DECAY*s + (B-s)*relu(exc) - (C+s)*relu(inh)) / TAU, masked on actions.

Sharding: pure data-parallel over the agent axis across 8 NeuronCores.
Per core: 32768 agents. Macro-tile = 128 partitions x G agents/partition.
Agent a (within a macro block) = p*G + g  (partition-major) so each
partition's HBM reads are contiguous.

Layout per macro-tile:
  wbuf  (128, 2*G*289): [W_pos g=0..G-1 | W_neg g=0..G-1], each row-major 17x17
  s2    (128, 2*G*17):  state duplicated twice (for pos/neg halves)
  prod = wbuf * broadcast(s)   (DVE tensor_tensor, in1 stride-0 on i axis)
  mv   = segmented reduce_add over inner 17 (DVE tensor_reduce axis=X)
  then gates/env/lateral (ACT + GPSIMD small ops), combine, mask, DMA out.
"""

import numpy as np

import concourse.bass as bass
import concourse.bacc as bacc
import concourse.mybir as mybir
from concourse.tile import TileContext
from concourse.bass_utils import run_bass_kernel_spmd

P = 128
N = 17
NN = N * N
NCORES = 8
B_TOTAL = 262144
B_CORE = B_TOTAL // NCORES  # 32768
G = 16                      # agents per partition per macro-tile
MACROS = B_CORE // (P * G)  # 16

FP = mybir.dt.float32
FH = mybir.dt.float16
AX = mybir.AxisListType
OP = mybir.AluOpType
AF = mybir.ActivationFunctionType

# Grossberg constants
TAU, DECAY, B_CAP, C_FLOOR = 0.8, 0.15, 1.0, 0.1
LAT_INHIB, DIV_SIGMA = 3.0, 0.3
ALPHA, BETA = 1.5, 0.75
INV_TAU = 1.0 / TAU                       # 1.25
U_BIAS = DECAY * INV_TAU                  # 0.1875 ; dS = R_e - 0.1*R_i - s*(U_BIAS + R_e + R_i)
LAT_DEN_C = DIV_SIGMA + 1e-6              # 0.300001


def build_program():
    nc = bacc.Bacc()
    st_d = nc.dram_tensor("state", [B_CORE, N], FP, kind="ExternalInput")
    wp_d = nc.dram_tensor("w_pos", [B_CORE, N, N], FH, kind="ExternalInput")
    wn_d = nc.dram_tensor("w_neg", [B_CORE, N, N], FH, kind="ExternalInput")
    fs_d = nc.dram_tensor("feas", [B_CORE, 4], FP, kind="ExternalInput")
    pt_d = nc.dram_tensor("pert", [B_CORE, N], FP, kind="ExternalInput")
    sh_d = nc.dram_tensor("state_h", [B_CORE, N], FH, kind="ExternalInput")
    out_d = nc.dram_tensor("out", [B_CORE, N], FP, kind="ExternalOutput")

    # (MACROS, 128, G*…) views, partition-major agent mapping
    wp_v = wp_d[:, :, :].rearrange("(m p g) i j -> m p (g i j)", p=P, g=G)
    wn_v = wn_d[:, :, :].rearrange("(m p g) i j -> m p (g i j)", p=P, g=G)
    st_v = st_d[:, :].rearrange("(m p g) n -> m p (g n)", p=P, g=G)
    pt_v = pt_d[:, :].rearrange("(m p g) n -> m p (g n)", p=P, g=G)
    sh_v = sh_d[:, :].rearrange("(m p g) n -> m p (g n)", p=P, g=G)
    fs_v = fs_d[:, :].rearrange("(m p g) f -> m p (g f)", p=P, g=G)
    out_v = out_d[:, :].rearrange("(m p g) n -> m p (g n)", p=P, g=G)

    GN = G * N
    with TileContext(nc) as tc:
        with (
            tc.tile_pool(name="big2", bufs=2) as pool2,
            tc.tile_pool(name="big1", bufs=1) as pool1,
        ):
            for m in range(MACROS):
                # ---- loads ----
                wbuf = pool2.tile([P, 2 * G * NN], FH, tag="wbuf")
                nc.sync.dma_start(out=wbuf[:, 0 : G * NN], in_=wp_v[m])
                nc.sync.dma_start(out=wbuf[:, G * NN :], in_=wn_v[m])
                s2 = pool2.tile([P, 2 * GN], FP, tag="s2")
                nc.sync.dma_start(out=s2[:, 0:GN], in_=st_v[m])
                nc.sync.dma_start(out=s2[:, GN:], in_=st_v[m])
                s2h = pool2.tile([P, 2 * GN], FH, tag="s2h")
                nc.sync.dma_start(out=s2h[:, 0:GN], in_=sh_v[m])
                nc.sync.dma_start(out=s2h[:, GN:], in_=sh_v[m])
                pert = pool2.tile([P, GN], FP, tag="pert")
                nc.sync.dma_start(out=pert[:], in_=pt_v[m])
                feas = pool2.tile([P, G * 4], FP, tag="feas")
                nc.sync.dma_start(out=feas[:], in_=fs_v[m])

                # ---- big multiply + segmented reduce (DVE) ----
                prod = pool1.tile([P, 2 * G * NN], FH, tag="prod")
                w4 = wbuf.rearrange("p (k i j) -> p k i j", i=N, j=N)
                p4 = prod.rearrange("p (k i j) -> p k i j", i=N, j=N)
                s4 = s2h.rearrange("p (k j) -> p k j", j=N)[:, :, None, :].broadcast_to(
                    [P, 2 * G, N, N]
                )
                nc.vector.tensor_tensor(out=p4, in0=w4, in1=s4, op=OP.mult)
                mv = pool2.tile([P, 2 * GN], FP, tag="mv")
                nc.vector.tensor_reduce(
                    out=mv[:],
                    in_=prod.rearrange("p (k j) -> p k j", j=N),
                    axis=AX.X,
                    op=OP.add,
                )
                mv3 = mv.rearrange("p (k n) -> p k n", n=N)

                # ---- gates (valence-controlled sigmoid on action rows) ----
                s3 = s2.rearrange("p (k n) -> p k n", n=N)
                pt3 = pert.rearrange("p (g n) -> p g n", n=N)
                ve = pool2.tile([P, 4 * G], FP, tag="ve")
                ve3 = ve.rearrange("p (g f) -> p g f", f=4)
                nc.gpsimd.tensor_tensor(
                    out=ve3, in0=s3[:, 0:G, 13:17], in1=pt3[:, :, 13:17], op=OP.add
                )
                ge = pool2.tile([P, 4 * G], FP, tag="ge")
                nc.scalar.activation(ge[:], ve[:], AF.Sigmoid, scale=ALPHA)
                gi = pool2.tile([P, 4 * G], FP, tag="gi")
                nc.scalar.activation(gi[:], ve[:], AF.Sigmoid, scale=-BETA)
                ge3 = ge.rearrange("p (g f) -> p g f", f=4)
                gi3 = gi.rearrange("p (g f) -> p g f", f=4)
                nc.gpsimd.tensor_tensor(
                    out=mv3[:, 0:G, 9:13], in0=mv3[:, 0:G, 9:13], in1=ge3, op=OP.mult
                )
                nc.gpsimd.tensor_tensor(
                    out=mv3[:, G : 2 * G, 9:13],
                    in0=mv3[:, G : 2 * G, 9:13],
                    in1=gi3,
                    op=OP.mult,
                )

                # ---- environmental drive on the 9 need rows ----
                reluP = pool2.tile([P, GN], FP, tag="reluP")
                nc.scalar.activation(reluP[:], pert[:], AF.Relu)
                reluN = pool2.tile([P, GN], FP, tag="reluN")
                nc.scalar.activation(reluN[:], pert[:], AF.Relu, scale=-1.0)
                rP3 = reluP.rearrange("p (g n) -> p g n", n=N)
                rN3 = reluN.rearrange("p (g n) -> p g n", n=N)
                nc.gpsimd.tensor_tensor(
                    out=mv3[:, 0:G, 0:9], in0=mv3[:, 0:G, 0:9], in1=rP3[:, :, 0:9], op=OP.add
                )
                nc.gpsimd.tensor_tensor(
                    out=mv3[:, G : 2 * G, 0:9],
                    in0=mv3[:, G : 2 * G, 0:9],
                    in1=rN3[:, :, 0:9],
                    op=OP.add,
                )

                # ---- lateral inhibition among the 4 action rows ----
                # all on GPSIMD to avoid cross-engine sync-wait overflow
                a01 = pool2.tile([P, 2 * G], FP, tag="a01")
                a013 = a01.rearrange("p (g f) -> p g f", f=2)
                nc.gpsimd.tensor_tensor(
                    out=a013, in0=s3[:, 0:G, 9:11], in1=s3[:, 0:G, 11:13], op=OP.add
                )
                suma = pool2.tile([P, G], FP, tag="suma")
                nc.gpsimd.tensor_tensor(
                    out=suma[:, :, None],
                    in0=a013[:, :, 0:1],
                    in1=a013[:, :, 1:2],
                    op=OP.add,
                )
                other = pool2.tile([P, 4 * G], FP, tag="other")
                other3 = other.rearrange("p (g f) -> p g f", f=4)
                nc.gpsimd.tensor_tensor(
                    out=other3,
                    in0=suma[:, :, None].broadcast_to([P, G, 4]),
                    in1=s3[:, 0:G, 9:13],
                    op=OP.subtract,
                )
                den = pool2.tile([P, 4 * G], FP, tag="den")
                nc.vector.tensor_scalar_add(out=den[:], in0=other[:], scalar1=LAT_DEN_C)
                recip = pool2.tile([P, 4 * G], FP, tag="recip")
                nc.vector.reciprocal(recip[:], den[:])
                lat = pool2.tile([P, 4 * G], FP, tag="lat")
                nc.vector.scalar_tensor_tensor(
                    out=lat[:],
                    in0=other[:],
                    scalar=LAT_INHIB,
                    in1=recip[:],
                    op0=OP.mult,
                    op1=OP.mult,
                )
                lat3 = lat.rearrange("p (g f) -> p g f", f=4)
                nc.gpsimd.tensor_tensor(
                    out=mv3[:, G : 2 * G, 9:13],
                    in0=mv3[:, G : 2 * G, 9:13],
                    in1=lat3,
                    op=OP.add,
                )

                # ---- shunting combine: dS = R_e - 0.1*R_i - s*(U_BIAS + R_e + R_i)
                # with R = relu(mv * 1.25) (scale folds through relu)
                r = pool2.tile([P, 2 * GN], FP, tag="r")
                nc.scalar.activation(r[:], mv[:], AF.Relu, scale=INV_TAU)
                t1 = pool1.tile([P, GN], FP, tag="t1")
                nc.gpsimd.tensor_tensor(
                    out=t1[:], in0=r[:, 0:GN], in1=r[:, GN:], op=OP.add
                )
                u = pool1.tile([P, GN], FP, tag="u")
                nc.vector.scalar_tensor_tensor(
                    out=u[:],
                    in0=t1[:],
                    scalar=U_BIAS,
                    in1=s2[:, 0:GN],
                    op0=OP.add,
                    op1=OP.mult,
                )
                v = pool1.tile([P, GN], FP, tag="v")
                nc.vector.scalar_tensor_tensor(
                    out=v[:],
                    in0=r[:, GN:],
                    scalar=-C_FLOOR,
                    in1=r[:, 0:GN],
                    op0=OP.mult,
                    op1=OP.add,
                )
                ob = pool2.tile([P, GN], FP, tag="ob")
                nc.gpsimd.tensor_tensor(out=ob[:], in0=v[:], in1=u[:], op=OP.subtract)
                ob3 = ob.rearrange("p (g n) -> p g n", n=N)
                fs3 = feas.rearrange("p (g f) -> p g f", f=4)
                nc.gpsimd.tensor_tensor(
                    out=ob3[:, :, 9:13], in0=ob3[:, :, 9:13], in1=fs3, op=OP.mult
                )

                nc.sync.dma_start(out=out_v[m], in_=ob[:])
    if not nc.is_finalized():
        nc.finalize()
    return nc


def make_in_maps(state, w_pos, w_neg, feasibility, perturbation):
    state = np.ascontiguousarray(np.asarray(state, dtype=np.float32))
    w_pos = np.ascontiguousarray(np.asarray(w_pos, dtype=np.float32))
    w_neg = np.ascontiguousarray(np.asarray(w_neg, dtype=np.float32))
    feas = np.ascontiguousarray(np.asarray(feasibility, dtype=np.float32))
    pert = np.ascontiguousarray(np.asarray(perturbation, dtype=np.float32))
    state_h = state.astype(np.float16)
    w_pos_h = np.ascontiguousarray(w_pos.astype(np.float16))
    w_neg_h = np.ascontiguousarray(w_neg.astype(np.float16))
    in_maps = []
    for c in range(NCORES):
        sl = slice(c * B_CORE, (c + 1) * B_CORE)
        in_maps.append(
            {
                "state": state[sl],
                "state_h": state_h[sl],
                "w_pos": w_pos_h[sl],
                "w_neg": w_neg_h[sl],
                "feas": feas[sl],
                "pert": pert[sl],
            }
        )
    return in_maps


def gather(results):
    return np.concatenate([r["out"] for r in results], axis=0)


def kernel(t=None, state=None, W_pos=None, W_neg=None, feasibility=None, perturbation=None, **_):
    nc = build_program()
    in_maps = make_in_maps(state, W_pos, W_neg, feasibility, perturbation)
    res = run_bass_kernel_spmd(nc, in_maps, list(range(NCORES)))
    return gather(res.results)


if __name__ == "__main__":
    rng = np.random.default_rng(0)
    inputs = {
        "t": rng.standard_normal(1).astype(np.float32),
        "state": rng.random((B_TOTAL, N), dtype=np.float32),
        "W_pos": rng.random((B_TOTAL, N, N), dtype=np.float32),
        "W_neg": rng.random((B_TOTAL, N, N), dtype=np.float32),
        "feasibility": rng.random((B_TOTAL, 4), dtype=np.float32),
        "perturbation": rng.standard_normal((B_TOTAL, N)).astype(np.float32),
    }
    out = kernel(**inputs)
    print(out.shape, out.dtype)



# revision 11
# speedup vs baseline: 1.6570x; 1.6570x over previous
"""Trainium2 Bass kernel for Grossberg dynamics (batched gated 17x17 matvecs).

dS/dt = (-DECAY*s + (B-s)*relu(exc) - (C+s)*relu(inh)) / TAU, masked on actions.

Sharding: pure data-parallel over the agent axis across 8 NeuronCores
(32768 agents per core).

Per-core algorithm (v2 — PE-assisted reduce):
  W is host-packed fp16 with the contraction axis j on SBUF partitions:
  partitions = (block b in 0..6, j in 0..16) = 119 rows; free axis =
  (pair in {pos,neg}, i, agent g). Then per macro-tile:
    1. DVE: prod = wt * broadcast(s_t)   -- single tensor_tensor in 2x mode
    2. PE : segmented sum over j via block-diagonal-ones matmuls:
            stationary = 128-column chunks of prod [119, 128],
            moving     = indicator ones [119, 7] (col b' is 1 on block b'),
            out[m, b'] = sum_j prod[(b', j), m]  -> PSUM fp32 [128, 7]
       This lands the matvec results re-spread across all 128 partitions
       (partition = position within the 128-column chunk).
    3. Epilogue (gates, env drive, lateral inhibition, shunting combine)
       on ACT/Pool/DVE, reading PSUM, writing fp16 dS, DMA out.

Main macros: A=512 agents/block, 7 blocks = 3584 agents x 9 macros = 32256.
Tail macro: 4 blocks x 128 agents = 512.  Total 32768 per core.

PSUM mv layout per macro: col = pair*512 + i*(GC*NB) + gc*NB + b
(one PSUM bank per pair), where agent-per-partition index a = (gc, b).
"""

import numpy as np

import concourse.bass as bass
import concourse.bacc as bacc
import concourse.mybir as mybir
from concourse.tile import TileContext
from concourse.bass_utils import run_bass_kernel_spmd

P = 128
N = 17
NCORES = 8
B_TOTAL = 262144
B_CORE = B_TOTAL // NCORES  # 32768

# main macro geometry
NB = 7          # blocks (j-groups) on partitions: 7*17 = 119
A = 512         # agents per block
GC = A // 128   # 128-col chunks per (pair, i): 4
AGENTS_MAIN = NB * A            # 3584
MACROS = 9                      # 9 * 3584 = 32256
# tail macro geometry
NB_T = 4
A_T = 128
GC_T = 1
AGENTS_TAIL = NB_T * A_T        # 512

FP = mybir.dt.float32
FH = mybir.dt.float16
AX = mybir.AxisListType
OP = mybir.AluOpType
AF = mybir.ActivationFunctionType

# Grossberg constants
TAU, DECAY, B_CAP, C_FLOOR = 0.8, 0.15, 1.0, 0.1
LAT_INHIB, DIV_SIGMA = 3.0, 0.3
ALPHA, BETA = 1.5, 0.75
INV_TAU = 1.0 / TAU                       # 1.25
U_BIAS = DECAY * INV_TAU                  # 0.1875 ; dS = Re - 0.1*Ri - s*(U_BIAS + Re + Ri)
LAT_DEN_C = DIV_SIGMA + 1e-6              # 0.300001

NAUX = 38  # s(17) | pert(17) | feas(4), fp16, per agent


def _macro(nc, tc, pools, m, nb, a_blk, gcn, views, ones_tile, sfx=""):
    """Emit one macro-tile: nb blocks x a_blk agents, gcn = a_blk // 128."""
    wt_v, st_v, aux_v, out_v = views
    pool_w, pool_m, pool_s, pool_ps = pools
    nparts = nb * N                 # partitions used by the mult (j-layout)
    fw = 2 * N * a_blk              # free elems of wt/prod per partition
    napp = gcn * nb                 # agents per partition in epilogue layout
    nmm = 2 * N * gcn               # number of matmul chunks
    PH = 512                        # psum col offset of the neg pair

    # ---- loads ----
    wt = pool_w.tile([nparts, fw], FH, tag="wt" + sfx)
    nc.sync.dma_start(out=wt[:], in_=wt_v)
    st = pool_s.tile([nparts, a_blk], FH, tag="st" + sfx)
    nc.sync.dma_start(out=st[:], in_=st_v)
    aux = pool_s.tile([P, napp * NAUX], FH, tag="aux" + sfx)
    nc.sync.dma_start(out=aux[:], in_=aux_v)

    # ---- big multiply (DVE, 2x mode) ----
    prod = pool_m.tile([nparts, fw], FH, tag="prod" + sfx)
    s_b = st[:, None, None, :].broadcast_to([nparts, 2, N, a_blk])
    w4 = wt.rearrange("p (t i g) -> p t i g", t=2, i=N)
    p4 = prod.rearrange("p (t i g) -> p t i g", t=2, i=N)
    nc.vector.tensor_tensor(out=p4, in0=w4, in1=s_b, op=OP.mult)

    # ---- segmented j-reduce on the PE ----
    # chunk c = (pair, i, gc): stationary prod[:, 128c : 128c+128],
    # out psum cols [pair*PH + (i*gcn + gc)*nb, +nb)
    mv = pool_ps.tile([P, 1024], FP, tag="mv" + sfx)
    for t in range(2):
        for i in range(N):
            for gc in range(gcn):
                c = (t * N + i) * gcn + gc
                off = t * PH + (i * gcn + gc) * nb
                nc.tensor.matmul(
                    mv[:, off : off + nb],
                    prod[:, 128 * c : 128 * (c + 1)],
                    ones_tile[:nparts, :nb],
                    start=True,
                    stop=True,
                )

    # epilogue APs --------------------------------------------------------
    # mv free layout per pair: [i: stride napp][a: stride 1], a = (gc, b)
    mv3 = mv.rearrange("p (t x) -> p t x", t=2)  # x in [0, 512)
    exc3 = mv3[:, 0, 0 : N * napp].rearrange("p (i a) -> p i a", i=N)
    inh3 = mv3[:, 1, 0 : N * napp].rearrange("p (i a) -> p i a", i=N)
    aux3 = aux.rearrange("p (a c) -> p a c", c=NAUX)
    s_T = aux3[:, :, 0:17]      # [p, a, c] agent-major
    pt_T = aux3[:, :, 17:34]
    fs_T = aux3[:, :, 34:38]

    # ---- gates: ve = s_v + p_v (agent-major), sigmoids on ACT (transpose) ----
    ve = pool_s.tile([P, napp * 4], FH, tag="ve" + sfx)
    ve3 = ve.rearrange("p (a r) -> p a r", r=4)
    nc.vector.tensor_tensor(out=ve3, in0=s_T[:, :, 13:17], in1=pt_T[:, :, 13:17], op=OP.add)
    # row-major [r, a] views of agent-major data
    veT = ve.rearrange("p (a r) -> p r a", r=4)
    ge = pool_s.tile([P, 4 * napp], FH, tag="ge" + sfx)
    ge3 = ge.rearrange("p (r a) -> p r a", a=napp)
    nc.scalar.activation(ge3, veT, AF.Sigmoid, scale=ALPHA)
    gi = pool_s.tile([P, 4 * napp], FH, tag="gi" + sfx)
    gi3 = gi.rearrange("p (r a) -> p r a", a=napp)
    nc.scalar.activation(gi3, veT, AF.Sigmoid, scale=-BETA)

    # ---- env drive relu(+-pert) on needs rows, row-major on ACT ----
    ptT9 = pt_T[:, :, 0:9].rearrange("p a i -> p i a")
    rp = pool_s.tile([P, 9 * napp], FH, tag="rp" + sfx)
    rp3 = rp.rearrange("p (i a) -> p i a", a=napp)
    nc.scalar.activation(rp3, ptT9, AF.Relu)
    rn = pool_s.tile([P, 9 * napp], FH, tag="rn" + sfx)
    rn3 = rn.rearrange("p (i a) -> p i a", a=napp)
    nc.scalar.activation(rn3, ptT9, AF.Relu, scale=-1.0)

    # ---- lateral inhibition among the 4 action rows ----
    sa = s_T[:, :, 9:13]
    t2 = pool_s.tile([P, napp * 2], FH, tag="t2" + sfx)
    t23 = t2.rearrange("p (a r) -> p a r", r=2)
    nc.vector.tensor_tensor(out=t23, in0=sa[:, :, 0:2], in1=sa[:, :, 2:4], op=OP.add)
    suma = pool_s.tile([P, napp], FH, tag="suma" + sfx)
    nc.vector.tensor_tensor(
        out=suma[:, :, None], in0=t23[:, :, 0:1], in1=t23[:, :, 1:2], op=OP.add
    )
    other = pool_s.tile([P, 4 * napp], FH, tag="other" + sfx)
    other3 = other.rearrange("p (r a) -> p r a", a=napp)
    saT = sa.rearrange("p a r -> p r a")
    nc.vector.tensor_tensor(
        out=other3,
        in0=suma[:, None, :].broadcast_to([P, 4, napp]),
        in1=saT,
        op=OP.subtract,
    )
    # den = (other + c)/3 so acr = 3/(c+other); lat = other * acr
    den = pool_s.tile([P, 4 * napp], FH, tag="den" + sfx)
    nc.gpsimd.tensor_scalar(
        out=den[:], in0=other[:], scalar1=LAT_DEN_C, scalar2=1.0 / LAT_INHIB,
        op0=OP.add, op1=OP.mult,
    )
    acr = pool_s.tile([P, 4 * napp], FH, tag="acr" + sfx)
    nc.vector.reciprocal(acr[:], den[:])
    lat = pool_s.tile([P, 4 * napp], FH, tag="lat" + sfx)
    nc.gpsimd.tensor_tensor(out=lat[:], in0=other[:], in1=acr[:], op=OP.mult)
    lat3 = lat.rearrange("p (r a) -> p r a", a=napp)

    # ---- apply gates / env / lateral to mv (PSUM RMW on DVE) ----
    nc.vector.tensor_tensor(out=exc3[:, 9:13], in0=exc3[:, 9:13], in1=ge3, op=OP.mult)
    nc.vector.tensor_tensor(out=inh3[:, 9:13], in0=inh3[:, 9:13], in1=gi3, op=OP.mult)
    nc.vector.tensor_tensor(out=exc3[:, 0:9], in0=exc3[:, 0:9], in1=rp3, op=OP.add)
    nc.vector.tensor_tensor(out=inh3[:, 0:9], in0=inh3[:, 0:9], in1=rn3, op=OP.add)
    nc.vector.tensor_tensor(out=inh3[:, 9:13], in0=inh3[:, 9:13], in1=lat3, op=OP.add)

    # ---- shunting combine: dS = Re - 0.1*Ri - s*(U_BIAS + Re + Ri) ----
    nmv = N * napp
    re = pool_s.tile([P, nmv], FH, tag="re" + sfx)
    nc.scalar.activation(re[:], mv3[:, 0, 0:nmv], AF.Relu, scale=INV_TAU)
    ri = pool_s.tile([P, nmv], FH, tag="ri" + sfx)
    nc.scalar.activation(ri[:], mv3[:, 1, 0:nmv], AF.Relu, scale=INV_TAU)

    sei = pool_s.tile([P, nmv], FH, tag="sei" + sfx)
    nc.vector.tensor_tensor(out=sei[:], in0=re[:], in1=ri[:], op=OP.add)
    su = pool_s.tile([P, nmv], FH, tag="su" + sfx)
    nc.gpsimd.tensor_scalar_add(out=su[:], in0=sei[:], scalar1=U_BIAS)
    sT_i = s_T.rearrange("p a i -> p i a")
    u = pool_s.tile([P, nmv], FH, tag="u" + sfx)
    nc.gpsimd.tensor_tensor(out=u[:], in0=su[:], in1=sT_i, op=OP.mult)
    rr = pool_s.tile([P, nmv], FH, tag="rr" + sfx)
    nc.gpsimd.tensor_scalar_mul(out=rr[:], in0=ri[:], scalar1=-C_FLOOR)
    v = pool_s.tile([P, nmv], FH, tag="v" + sfx)
    nc.gpsimd.tensor_tensor(out=v[:], in0=rr[:], in1=re[:], op=OP.add)
    ds = pool_s.tile([P, nmv], FH, tag="ds" + sfx)
    nc.gpsimd.tensor_tensor(out=ds[:], in0=v[:], in1=u[:], op=OP.subtract)
    ds3 = ds.rearrange("p (i a) -> p i a", a=napp)
    fsT = fs_T.rearrange("p a r -> p r a")
    nc.gpsimd.tensor_tensor(out=ds3[:, 9:13], in0=ds3[:, 9:13], in1=fsT, op=OP.mult)

    nc.sync.dma_start(out=out_v, in_=ds[:])


def build_program():
    nc = bacc.Bacc()
    wt_d = nc.dram_tensor("wt", [MACROS, NB * N, 2 * N * A], FH, kind="ExternalInput")
    st_d = nc.dram_tensor("st", [MACROS, NB * N, A], FH, kind="ExternalInput")
    aux_d = nc.dram_tensor("aux", [MACROS, P, GC * NB * NAUX], FH, kind="ExternalInput")
    out_d = nc.dram_tensor("out", [MACROS, P, N * GC * NB], FH, kind="ExternalOutput")
    wtt_d = nc.dram_tensor("wt_t", [NB_T * N, 2 * N * A_T], FH, kind="ExternalInput")
    stt_d = nc.dram_tensor("st_t", [NB_T * N, A_T], FH, kind="ExternalInput")
    auxt_d = nc.dram_tensor("aux_t", [P, GC_T * NB_T * NAUX], FH, kind="ExternalInput")
    outt_d = nc.dram_tensor("out_t", [P, N * GC_T * NB_T], FH, kind="ExternalOutput")
    ones_d = nc.dram_tensor("ones", [NB * N, NB], FH, kind="ExternalInput")
    onest_d = nc.dram_tensor("ones_tl", [NB_T * N, NB_T], FH, kind="ExternalInput")

    with TileContext(nc) as tc:
        with (
            nc.allow_low_precision(reason="fp16 pipeline; rel-err gate is 2e-2"),
            tc.tile_pool(name="pw", bufs=2) as pool_w,
            tc.tile_pool(name="pm", bufs=2) as pool_m,
            tc.tile_pool(name="ps", bufs=2) as pool_s,
            tc.tile_pool(name="pones", bufs=1) as pool_c,
            tc.tile_pool(name="ppsum", bufs=2, space="PSUM") as pool_ps,
        ):
            # block-indicator ones: [119, 7] (col b is 1 on partitions of
            # block b). Engine writes can't start at partition 17, so these
            # tiny constants come in via DMA.
            ones = pool_c.tile([NB * N, NB], FH, tag="ones")
            nc.sync.dma_start(out=ones[:], in_=ones_d[:, :])
            ones_t = pool_c.tile([NB_T * N, NB_T], FH, tag="ones_t")
            nc.sync.dma_start(out=ones_t[:], in_=onest_d[:, :])

            pools = (pool_w, pool_m, pool_s, pool_ps)
            for m in range(MACROS):
                _macro(
                    nc, tc, pools, m, NB, A, GC,
                    (wt_d[m], st_d[m], aux_d[m], out_d[m]),
                    ones,
                )
            _macro(
                nc, tc, pools, MACROS, NB_T, A_T, GC_T,
                (wtt_d[:, :], stt_d[:, :], auxt_d[:, :], outt_d[:, :]),
                ones_t,
                sfx="_t",
            )
    if not nc.is_finalized():
        nc.finalize()
    return nc


def make_in_maps(state, w_pos, w_neg, feasibility, perturbation):
    state = np.asarray(state, dtype=np.float32)
    feas = np.asarray(feasibility, dtype=np.float32)
    pert = np.asarray(perturbation, dtype=np.float32)
    s16 = state.astype(np.float16)
    a38 = np.concatenate(
        [s16, pert.astype(np.float16), feas.astype(np.float16)], axis=1
    )  # [B, 38]
    wall = np.stack(
        [np.asarray(w_pos, np.float32), np.asarray(w_neg, np.float32)], axis=1
    ).astype(np.float16)  # [B, 2, 17, 17]

    nmain = MACROS * AGENTS_MAIN
    in_maps = []
    for c in range(NCORES):
        sl = slice(c * B_CORE, (c + 1) * B_CORE)
        wc, sc, ac = wall[sl], s16[sl], a38[sl]
        # main: agent = m*3584 + b*512 + gc*128 + p
        wm = wc[:nmain].reshape(MACROS, NB, GC, P, 2, N, N)
        wt = np.ascontiguousarray(wm.transpose(0, 1, 6, 4, 5, 2, 3)).reshape(
            MACROS, NB * N, 2 * N * A
        )
        sm = sc[:nmain].reshape(MACROS, NB, GC, P, N)
        st = np.ascontiguousarray(sm.transpose(0, 1, 4, 2, 3)).reshape(MACROS, NB * N, A)
        am = ac[:nmain].reshape(MACROS, NB, GC, P, NAUX)
        aux = np.ascontiguousarray(am.transpose(0, 3, 2, 1, 4)).reshape(
            MACROS, P, GC * NB * NAUX
        )
        # tail: agent = nmain + b*128 + p
        wtl = wc[nmain:].reshape(NB_T, P, 2, N, N)
        wt_t = np.ascontiguousarray(wtl.transpose(0, 4, 2, 3, 1)).reshape(
            NB_T * N, 2 * N * A_T
        )
        stl = sc[nmain:].reshape(NB_T, P, N)
        st_t = np.ascontiguousarray(stl.transpose(0, 2, 1)).reshape(NB_T * N, A_T)
        atl = ac[nmain:].reshape(NB_T, P, NAUX)
        aux_t = np.ascontiguousarray(atl.transpose(1, 0, 2)).reshape(P, NB_T * NAUX)
        ones = np.kron(np.eye(NB, dtype=np.float16), np.ones((N, 1), np.float16))
        ones_t = np.kron(np.eye(NB_T, dtype=np.float16), np.ones((N, 1), np.float16))
        in_maps.append(
            {
                "wt": wt, "st": st, "aux": aux,
                "wt_t": wt_t, "st_t": st_t, "aux_t": aux_t,
                "ones": ones, "ones_tl": ones_t,
            }
        )
    return in_maps


def gather(results):
    outs = []
    for r in results:
        o = np.asarray(r["out"])  # [9, 128, 476] fp16
        om = o.reshape(MACROS, P, N, GC, NB).transpose(0, 4, 3, 1, 2)
        om = om.reshape(MACROS * AGENTS_MAIN, N)
        ot = np.asarray(r["out_t"]).reshape(P, N, GC_T, NB_T).transpose(2, 3, 0, 1)
        ot = ot.reshape(AGENTS_TAIL, N)
        outs.append(np.concatenate([om, ot], axis=0))
    return np.concatenate(outs, axis=0).astype(np.float32)


def kernel(t=None, state=None, W_pos=None, W_neg=None, feasibility=None, perturbation=None, **_):
    nc = build_program()
    in_maps = make_in_maps(state, W_pos, W_neg, feasibility, perturbation)
    res = run_bass_kernel_spmd(nc, in_maps, list(range(NCORES)))
    return gather(res.results)


if __name__ == "__main__":
    rng = np.random.default_rng(0)
    inputs = {
        "t": rng.standard_normal(1).astype(np.float32),
        "state": rng.random((B_TOTAL, N), dtype=np.float32),
        "W_pos": rng.random((B_TOTAL, N, N), dtype=np.float32),
        "W_neg": rng.random((B_TOTAL, N, N), dtype=np.float32),
        "feasibility": rng.random((B_TOTAL, 4), dtype=np.float32),
        "perturbation": rng.standard_normal((B_TOTAL, N)).astype(np.float32),
    }
    out = kernel(**inputs)
    print(out.shape, out.dtype)
